# revision 58
# baseline (speedup 1.0000x reference)
"""Trainium2 Bass kernel for nn_EnhancedSpatioTemporalLayer, v7.

out = relu( sum_k T_k @ relu(conv1x3(x)+b) @ theta_k ) + x @ res_w.T + res_b
with T_0 = I, T_1 = L, T_2 = 2L^2 - I (cheb polys, L symmetric), refactored as

    t    = relu(conv(x) + conv_b)                  # [2*64, n, pair] bf16
    V    = t^T @ blockdiag(Th, Th)                 # [n, 384]: V_A|V_B per PAIR
    fin  = I @ Va + L @ Vb + L^2 @ Vc              # [n, slot, o]
    out  = relu(fin) + res,  res = x^T @ res_w.T + res_b

HW is matmul-issue bound (~110ns fixed + free/2.4GHz per MM; LDWEIGHTS is
not free and not well overlapped), so v7 minimizes MM count and pairs the
narrow-output MMs in disjoint PE column groups so they stream concurrently:

- V: one 384-free MM per (pair, n-half): both conv halves packed in a
  128-row stationary against blockdiag([Th, Th]) moving.
- chain: merged tail-contract stationaries [Lt;L2t] (84r) and
  [Lt';L2t';I42] (126r) via a cross-partition DMA restack of V-tail
  -> 7 chain MMs per 8 taus.
- tail-output MMs (ft chain, res-tail) land on partitions 64:106
  (tile_position (0,64), col groups 2-3) and are emitted right after
  V-tail MMs (col groups 0-1) -> concurrent streams.
- residual: 31-row 4-center patches, one stationary per superblock.
- evac: one 768-col PSUM->SBUF copy per pair (DVE/ACT alternating);
  relu 1024 (ACT) + add 1024 (DVE) per 8 taus.

Sharding: data-parallel over batch, 2 batches per core, 8 cores.
Output per (b, g): main [128 n, 9, 512] + tail [42 n @ parts 64:106, ...],
cols = (sblock, slot=2p+h, o), tau = h*TH + g*TC + 4*sb + p.
"""

import numpy as np
import ml_dtypes

B, FIN, N, T = 16, 3, 170, 288
H, O = 64, 64
NCORES = 8
BPC = B // NCORES       # batches per core
TH = T // 2             # tau half length (144)
NGRP = 4                # groups per half
TC = TH // NGRP         # 36 tau-pairs per group
NSB = TC // 4           # 9 superblocks (4 pairs = 8 taus) per group
NT = N - 128            # vertex tail size (42)

bf16 = ml_dtypes.bfloat16

_cache = {}


def _build(reps=1, timing=False):
    import concourse.mybir as mybir
    import concourse.tile as tile
    from concourse import bacc

    f32 = mybir.dt.float32
    bft = mybir.dt.bfloat16
    Relu = mybir.ActivationFunctionType.Relu
    ADD = mybir.AluOpType.add

    nc = bacc.Bacc(None, target_bir_lowering=False)
    if timing:
        x_d = nc.dram_tensor("x", [BPC, NGRP, 31, TC, N], bft,
                             kind="Internal").ap()
    else:
        x_d = nc.declare_dram_parameter(
            "x", [BPC, NGRP, 31, TC, N], bft, isOutput=False)
    wc_d = nc.declare_dram_parameter("wc", [128, 2 * H], bft, isOutput=False)
    th_d = nc.declare_dram_parameter("th", [128, 384], bft, isOutput=False)
    rb_d = nc.declare_dram_parameter("rb", [128, 512], bft, isOutput=False)
    ln0_d = nc.declare_dram_parameter("ln0", [128, 128], bft, isOutput=False)
    l2n0_d = nc.declare_dram_parameter("l2n0", [128, 128], bft, isOutput=False)
    ln1_d = nc.declare_dram_parameter("ln1", [128, NT], bft, isOutput=False)
    l2n1_d = nc.declare_dram_parameter("l2n1", [128, NT], bft, isOutput=False)
    eyn0_d = nc.declare_dram_parameter("eyn0", [128, 128], bft, isOutput=False)
    lt2m_d = nc.declare_dram_parameter("lt2m", [2 * NT, 128], bft,
                                       isOutput=False)
    lt3t_d = nc.declare_dram_parameter("lt3t", [3 * NT, NT], bft,
                                       isOutput=False)
    cb_d = nc.declare_dram_parameter("cb", [128, 2], f32, isOutput=False)
    if timing:
        out_d = nc.dram_tensor("out", [BPC, NGRP, 128, NSB, 512], bft,
                               kind="Internal").ap()
        out2_d = nc.dram_tensor("out2", [BPC, NGRP, NT, NSB, 512], bft,
                                kind="Internal").ap()
        tick_d = nc.declare_dram_parameter("tick", [128, 2], f32, isOutput=True)
    else:
        out_d = nc.declare_dram_parameter("out", [BPC, NGRP, 128, NSB, 512],
                                          bft, isOutput=True)
        out2_d = nc.declare_dram_parameter("out2", [BPC, NGRP, NT, NSB, 512],
                                           bft, isOutput=True)

    with tile.TileContext(nc) as tc:
        with (
            tc.tile_pool(name="const", bufs=1) as cp,
            tc.tile_pool(name="xp", bufs=2) as xpp,
            tc.tile_pool(name="tsb", bufs=2) as tsp,
            tc.tile_pool(name="vsb", bufs=2) as vsbp,
            tc.tile_pool(name="vstk", bufs=2) as vskp,
            tc.tile_pool(name="osb", bufs=2) as osbp,
            tc.tile_pool(name="rel", bufs=2) as relp,
            tc.tile_pool(name="vps", bufs=2, space="PSUM") as vpp,
            tc.tile_pool(name="finA", bufs=1, space="PSUM") as finAp,
            tc.tile_pool(name="finB", bufs=1, space="PSUM") as finBp,
            tc.tile_pool(name="resA", bufs=1, space="PSUM") as resAp,
            tc.tile_pool(name="resB", bufs=1, space="PSUM") as resBp,
        ):
            wc_t = cp.tile([128, 2 * H], bft)
            th_t = cp.tile([128, 384], bft)
            rb_t = cp.tile([128, 512], bft)
            ln0_t = cp.tile([128, 128], bft)
            l2n0_t = cp.tile([128, 128], bft)
            ln1_t = cp.tile([128, NT], bft)
            l2n1_t = cp.tile([128, NT], bft)
            eyn0_t = cp.tile([128, 128], bft)
            lt2m_t = cp.tile([2 * NT, 128], bft)
            lt3t_t = cp.tile([3 * NT, NT], bft)
            cb_t = cp.tile([128, 2], f32)
            for t_, d_ in ((wc_t, wc_d), (th_t, th_d), (rb_t, rb_d),
                           (ln0_t, ln0_d), (l2n0_t, l2n0_d), (ln1_t, ln1_d),
                           (l2n1_t, l2n1_d), (eyn0_t, eyn0_d),
                           (lt2m_t, lt2m_d), (lt3t_t, lt3t_d), (cb_t, cb_d)):
                nc.sync.dma_start(out=t_[:, :], in_=d_[:, :])

            def prep_tail(st):
                """tail-side tiles + narrow-output MM closures. These MMs
                output to partitions 64:106 (col groups 2-3) so each runs
                concurrent with the preceding V-tail MM (col groups 0-1)."""
                sb, vsb, vstk, xp, t_sb, osb = st
                q0 = 4 * sb
                finB = finBp.tile([128, 512], f32, name="finB")
                resB = resBp.tile([128, 512], f32, name="resB")
                rel = relp.tile([128, 1024], bft, name="rel")
                st.extend([finB, resB, rel])
                ft = finB[64:64 + NT, :].rearrange("n (s c) -> n s c", s=8)

                def m_restail():
                    nc.tensor.matmul(resB[64:64 + NT, :],
                                     xp[0:31, q0, 128:N],
                                     rb_t[0:31, :], start=True, stop=True)

                def m_ftmt():
                    nc.tensor.matmul(ft[:, :, :], lt3t_t[0:3 * NT, 0:NT],
                                     vstk[0:3 * NT, :, :],
                                     start=True, stop=False,
                                     skip_group_check=True)

                def m_ftl():
                    nc.tensor.matmul(ft[:, :, :], ln1_t[:, 0:NT],
                                     vsb[:, :, 0, :, 64:128],
                                     start=False, stop=False,
                                     skip_group_check=True)

                def m_ftl2():
                    nc.tensor.matmul(ft[:, :, :], l2n1_t[:, 0:NT],
                                     vsb[:, :, 0, :, 128:192],
                                     start=False, stop=True,
                                     skip_group_check=True)

                return [m_restail, m_ftmt, m_ftl, m_ftl2]

            def st2a(st):
                """finA/resA alloc + 4 full-width fm chain MMs."""
                sb, vsb, vstk, xp, t_sb, osb = st[:6]
                finA = finAp.tile([128, 512], f32, name="finA")
                resA = resAp.tile([128, 512], f32, name="resA")
                st.extend([finA, resA])
                fm = finA[:, :].rearrange("n (s c) -> n s c", s=8)
                nc.tensor.matmul(fm[:, :, :], ln0_t[:, :],
                                 vsb[:, :, 0, :, 64:128],
                                 start=True, stop=False, skip_group_check=True)
                nc.tensor.matmul(fm[:, :, :], l2n0_t[:, :],
                                 vsb[:, :, 0, :, 128:192],
                                 start=False, stop=False, skip_group_check=True)
                nc.tensor.matmul(fm[:, :, :], lt2m_t[0:2 * NT, :],
                                 vstk[0:2 * NT, :, :],
                                 start=False, stop=False, skip_group_check=True)
                nc.tensor.matmul(fm[:, :, :], eyn0_t[:, :],
                                 vsb[:, :, 0, :, 0:64],
                                 start=False, stop=True, skip_group_check=True)

            def finishB(st):
                """tail relu + add: frees finB/resB."""
                sb = st[0]
                osb, finB, resB, rel = st[5], st[6], st[7], st[8]
                nc.scalar.activation(rel[64:64 + NT, 512:1024],
                                     finB[64:64 + NT, :], Relu)
                nc.vector.tensor_tensor(osb[64:64 + NT, sb, 512:1024],
                                        rel[64:64 + NT, 512:1024],
                                        resB[64:64 + NT, :], ADD)

            def finishA(st):
                """res-main MM + main relu + add: frees finA/resA."""
                sb, vsb, vstk, xp, t_sb, osb = st[:6]
                rel, finA, resA = st[8], st[9], st[10]
                q0 = 4 * sb
                nc.tensor.matmul(resA[:, :], xp[0:31, q0, 0:128],
                                 rb_t[0:31, :], start=True, stop=True)
                nc.scalar.activation(rel[:, 0:512], finA[:, :], Relu)
                nc.vector.tensor_tensor(osb[:, sb, 0:512], rel[:, 0:512],
                                        resA[:, :], ADD)

            def body():
                for b in range(BPC):
                    for g in range(NGRP):
                        xp = xpp.tile([128, TC, N], bft, name="xp")
                        nc.sync.dma_start(out=xp[0:31, :, :],
                                          in_=x_d[b, g, :, :, :])
                        t_sb = tsp.tile([128, TC, N], bft, name="t_sb")
                        for jj in range(TC // 6):
                            craw = vpp.tile([128, 2, 512], f32, name="vps")
                            for jh in range(2):
                                j = 2 * jj + jh
                                cps = craw[:, jh, 0:510]
                                cps = cps.rearrange("n (t c) -> n t c", t=3)
                                nc.tensor.matmul(cps[:, :, :], wc_t[0:18, :],
                                                 xp[0:18, 3 * j:3 * j + 3, :],
                                                 start=True, stop=True)
                            cin = craw[:, :, 0:510]
                            nc.scalar.activation(
                                t_sb[:, 6 * jj:6 * jj + 6, :].rearrange(
                                    "n t c -> n (t c)").rearrange(
                                    "n (j c) -> n j c", j=2),
                                cin, Relu, bias=cb_t[:, 0:1], scale=1.0)
                        osb = osbp.tile([128, NSB, 1024], bft, name="osb")
                        pend = None
                        tq = None
                        for sb in range(NSB + 1):
                            if pend is not None:
                                tq = prep_tail(pend)
                            if sb == NSB:
                                if pend is not None:
                                    tq[0]()
                                    tq[1]()
                                    st2a(pend)
                                    tq[2]()
                                    tq[3]()
                                    finishB(pend)
                                    finishA(pend)
                                break
                            q0 = 4 * sb
                            vsb = vsbp.tile([128, 4, 2, 2, 192], bft,
                                            name="vsb")
                            vstk = vskp.tile([128, 8, 64], bft, name="vstk")
                            for p in range(4):
                                vps = vpp.tile([128, 2, 512], f32, name="vps")
                                nc.tensor.matmul(
                                    vps[:, 0, 0:384],
                                    t_sb[0:128, q0 + p, 0:128],
                                    th_t[0:128, :],
                                    start=True, stop=True)
                                nc.tensor.matmul(
                                    vps[0:NT, 1, 0:384],
                                    t_sb[0:128, q0 + p, 128:N],
                                    th_t[0:128, :],
                                    start=True, stop=True)
                                if pend is not None:
                                    tq[p]()
                                vv = vps[:, :, 0:384].rearrange(
                                    "n m (h c) -> n m h c", h=2)
                                if p in (0, 3):
                                    nc.vector.tensor_copy(vsb[:, p, :, :, :],
                                                          vv[:, :, :, :])
                                else:
                                    nc.scalar.copy(vsb[:, p, :, :, :],
                                                   vv[:, :, :, :])
                                if p == 1 and pend is not None:
                                    st2a(pend)
                            # re-partition V-tail: [Vb_t; Vc_t; Va_t] stacked
                            # on partitions 0:126 for merged tail contracts
                            for ci, c0 in ((0, 64), (1, 128), (2, 0)):
                                for h in range(2):
                                    nc.gpsimd.dma_start(
                                        out=vstk[NT * ci:NT * ci + NT,
                                                 h::2, :],
                                        in_=vsb[0:NT, :, 1, h, c0:c0 + 64])
                            if pend is not None:
                                finishB(pend)
                                finishA(pend)
                            pend = [sb, vsb, vstk, xp, t_sb, osb]
                        nc.gpsimd.dma_start(
                            out=out_d[b, g, :, :, :],
                            in_=osb[:, :, 0:512])
                        nc.gpsimd.dma_start(
                            out=out2_d[b, g, :, :, :],
                            in_=osb[64:64 + NT, :, 512:1024])

            if reps > 1:
                with tc.For_i(0, reps, 1):
                    body()
            else:
                body()
            if timing:
                nc.sync.dma_start(out=tick_d[:, :], in_=cb_t[:, :])

    nc.compile()
    return nc


def _prep(inputs):
    cheb = np.asarray(inputs["cheb"], np.float32)
    conv_w = np.asarray(inputs["conv_w"], np.float32)
    conv_b = np.asarray(inputs["conv_b"], np.float32)
    theta = np.asarray(inputs["theta"], np.float32)
    res_w = np.asarray(inputs["res_w"], np.float32)
    res_b = np.asarray(inputs["res_b"], np.float32)

    L = cheb[1]
    L2 = (cheb[2] + np.eye(N, dtype=np.float32)) / 2.0

    # block-diagonal conv weights (rows: [s1,s0,s2] x FIN, A cols 0:64,
    # B cols 64:128)
    wc = np.zeros((128, 2 * H), bf16)
    for slot, s in ((0, 1), (1, 0), (2, 2)):
        for f in range(FIN):
            wc[3 * slot + f, 0:H] = conv_w[:, f, 0, s].astype(bf16)
            wc[9 + 3 * slot + f, H:2 * H] = conv_w[:, f, 0, s].astype(bf16)

    # block-diagonal theta: rows 0:64 (A half) -> cols 0:192, rows 64:128
    # (B half) -> cols 192:384; col layout per half: [th0-th2 | th1 | 2*th2]
    th = np.zeros((128, 384), bf16)
    blk = np.concatenate([(theta[0] - theta[2]), theta[1],
                          2.0 * theta[2]], axis=1).astype(bf16)   # [64, 192]
    th[0:64, 0:192] = blk
    th[64:128, 192:384] = blk

    # residual rhs: fin col order slot=2p+h; center of pair p half A/B sits
    # at xp patch rows below; bias via ones row 30
    rb = np.zeros((128, 512), bf16)
    rwt = res_w.T.astype(bf16)
    rows = {(0, 0): 0, (0, 1): 9, (1, 0): 6, (1, 1): 15,
            (2, 0): 18, (2, 1): 24, (3, 0): 21, (3, 1): 27}
    for p in range(4):
        for h in range(2):
            slot = 2 * p + h
            r0 = rows[(p, h)]
            rb[r0:r0 + 3, 64 * slot:64 * slot + 64] = rwt
    for slot in range(8):
        rb[30, 64 * slot:64 * slot + 64] = res_b.astype(bf16)

    w = {
        "wc": wc, "th": th, "rb": rb,
        "ln0": L[0:128, 0:128].astype(bf16),
        "l2n0": L2[0:128, 0:128].astype(bf16),
        "ln1": L[0:128, 128:N].astype(bf16),
        "l2n1": L2[0:128, 128:N].astype(bf16),
        "eyn0": np.eye(128, dtype=bf16),
        "lt2m": np.concatenate(
            [L[128:N, 0:128], L2[128:N, 0:128]], axis=0).astype(bf16),
        "lt3t": np.concatenate(
            [L[128:N, 128:N], L2[128:N, 128:N],
             np.eye(NT, dtype=np.float32)], axis=0).astype(bf16),
    }
    cb = np.zeros((128, 2), np.float32)
    cb[0:64, 0] = conv_b
    cb[64:128, 0] = conv_b
    w["cb"] = cb
    return w


def _prep_x(x):
    """[B, FIN, N, T] -> [B, NGRP, 31, TC, N] bf16 per-group patch blocks.

    rows 0:9 A-half conv patch [s1, s0, s2] x FIN, rows 9:18 B-half,
    rows 18:24 [s3, s4] x FIN A-half, 24:30 B-half, row 30 ones.
    Zero-padded at global tau edges."""
    Bn = x.shape[0]
    xt = np.transpose(x, (0, 1, 3, 2))          # [B, FIN, T, N]
    xprep = np.zeros((Bn, 15, T, N), np.float32)
    xprep[:, 0:3] = xt                          # s1 = x[t]
    xprep[:, 3:6, 1:T] = xt[:, :, 0:T - 1]      # s0 = x[t-1]
    xprep[:, 6:9, 0:T - 1] = xt[:, :, 1:T]      # s2 = x[t+1]
    xprep[:, 9:12, 0:T - 2] = xt[:, :, 2:T]     # s3 = x[t+2]
    xprep[:, 12:15, 0:T - 3] = xt[:, :, 3:T]    # s4 = x[t+3]
    xall = np.zeros((Bn, NGRP, 31, TC, N), np.float32)
    xall[:, :, 30] = 1.0
    for g in range(NGRP):
        tbA = g * TC
        tbB = tbA + TH
        xall[:, g, 0:9] = xprep[:, 0:9, tbA:tbA + TC]
        xall[:, g, 9:18] = xprep[:, 0:9, tbB:tbB + TC]
        xall[:, g, 18:24] = xprep[:, 9:15, tbA:tbA + TC]
        xall[:, g, 24:30] = xprep[:, 9:15, tbB:tbB + TC]
    return xall.astype(bf16)


def _unshard_out(main, tail):
    """main [BPC, NGRP, 128, NSB, 512], tail [.., NT, ..] -> [BPC,O,N,T]"""
    out = np.empty((BPC, O, N, T), np.float32)
    main = np.asarray(main).astype(np.float32)
    tail = np.asarray(tail).astype(np.float32)
    for part, n0, n1 in ((main, 0, 128), (tail, 128, N)):
        r = part.reshape(BPC, NGRP, n1 - n0, NSB, 4, 2, O)
        # dims: b, g, n, sb, p, h, o ; tau = h*TH + g*TC + 4*sb + p
        r = np.transpose(r, (0, 6, 2, 5, 1, 3, 4))  # b, o, n, h, g, sb, p
        out[:, :, n0:n1, :] = r.reshape(BPC, O, n1 - n0, T)
    return out


def kernel(**inputs):
    from concourse.bass_utils import run_bass_kernel_spmd

    if "nc" not in _cache:
        _cache["nc"] = _build(1)
    nc = _cache["nc"]

    x = np.asarray(inputs["x"], np.float32)
    weights = _prep(inputs)
    xprep = _prep_x(x)
    in_maps = []
    for cid in range(NCORES):
        m = dict(weights)
        m["x"] = np.ascontiguousarray(xprep[cid * BPC:(cid + 1) * BPC])
        in_maps.append(m)
    res = run_bass_kernel_spmd(nc, in_maps, list(range(NCORES)), trace=False)
    out = np.concatenate(
        [_unshard_out(res.results[cid]["out"], res.results[cid]["out2"])
         for cid in range(NCORES)], axis=0)
    return out.astype(np.float32)


# revision 64
# speedup vs baseline: 1.2521x; 1.2521x over previous
"""Trainium2 Bass kernel for nn_EnhancedSpatioTemporalLayer, v7.

out = relu( sum_k T_k @ relu(conv1x3(x)+b) @ theta_k ) + x @ res_w.T + res_b
with T_0 = I, T_1 = L, T_2 = 2L^2 - I (cheb polys, L symmetric), refactored as

    t    = relu(conv(x) + conv_b)                  # [2*64, n, pair] bf16
    V    = t^T @ blockdiag(Th, Th)                 # [n, 384]: V_A|V_B per PAIR
    fin  = I @ Va + L @ Vb + L^2 @ Vc              # [n, slot, o]
    out  = relu(fin) + res,  res = x^T @ res_w.T + res_b

HW is matmul-issue bound (~110ns fixed + free/2.4GHz per MM; LDWEIGHTS is
not free and poorly overlapped), so the kernel minimizes MM count:

- V: one 384-free MM per (pair, n-half): both conv halves packed in a
  128-row stationary against blockdiag([Th, Th]) moving -> 8 V MMs per
  8-tau superblock (vs 16 per-tau MMs).
- chain: merged tail-contract stationaries [Lt;L2t] (84 rows) and
  [Lt';L2t';I42] (126 rows) via a cross-partition DMA restack of the
  V-tail slots -> 7 chain MMs per 8 taus (vs 20 in v3).
- residual: 31-row 4-center patches (s1..s4), one stationary per
  superblock -> 2 MMs per 8 taus.
- evac: one 768-col PSUM->SBUF copy per pair (DVE/ACT alternating);
  relu 1024 cols (ACT) + add 1024 cols (DVE) per 8 taus; conv activation
  merged to 1020-col instructions.
- PSUM: V pair-tiles 2 banks x 2 bufs + fin 2 + res 2 (conv scratch
  shares the res pool) = 8 banks; chain/residual software-pipelined one
  superblock behind the V matmuls.

Sharding: data-parallel over batch, 2 batches per core, 8 cores.
Output per (b, g): main [128 n, 9, 512] + tail [42 n, 9, 512], cols =
(sblock, slot=2p+h, o), tau = h*TH + g*TC + 4*sb + p; host reassembles.
"""

import numpy as np
import ml_dtypes

B, FIN, N, T = 16, 3, 170, 288
H, O = 64, 64
NCORES = 8
BPC = B // NCORES       # batches per core
TH = T // 2             # tau half length (144)
NGRP = 4                # groups per half
TC = TH // NGRP         # 36 tau-pairs per group
NSB = TC // 4           # 9 superblocks (4 pairs = 8 taus) per group
NT = N - 128            # vertex tail size (42)

bf16 = ml_dtypes.bfloat16

_cache = {}


def _build(reps=1, timing=False):
    import concourse.mybir as mybir
    import concourse.tile as tile
    from concourse import bacc

    f32 = mybir.dt.float32
    bft = mybir.dt.bfloat16
    Relu = mybir.ActivationFunctionType.Relu
    ADD = mybir.AluOpType.add

    nc = bacc.Bacc(None, target_bir_lowering=False)
    if timing:
        x_d = nc.dram_tensor("x", [BPC, NGRP, 31, TC, N], bft,
                             kind="Internal").ap()
    else:
        x_d = nc.declare_dram_parameter(
            "x", [BPC, NGRP, 31, TC, N], bft, isOutput=False)
    wc_d = nc.declare_dram_parameter("wc", [128, 2 * H], bft, isOutput=False)
    th_d = nc.declare_dram_parameter("th", [128, 384], bft, isOutput=False)
    rb_d = nc.declare_dram_parameter("rb", [128, 512], bft, isOutput=False)
    ln0_d = nc.declare_dram_parameter("ln0", [128, 128], bft, isOutput=False)
    l2n0_d = nc.declare_dram_parameter("l2n0", [128, 128], bft, isOutput=False)
    ln1_d = nc.declare_dram_parameter("ln1", [128, NT], bft, isOutput=False)
    l2n1_d = nc.declare_dram_parameter("l2n1", [128, NT], bft, isOutput=False)
    eyn0_d = nc.declare_dram_parameter("eyn0", [128, 128], bft, isOutput=False)
    lt2m_d = nc.declare_dram_parameter("lt2m", [2 * NT, 128], bft,
                                       isOutput=False)
    lt3t_d = nc.declare_dram_parameter("lt3t", [3 * NT, NT], bft,
                                       isOutput=False)
    cb_d = nc.declare_dram_parameter("cb", [128, 2], f32, isOutput=False)
    if timing:
        out_d = nc.dram_tensor("out", [BPC, NGRP, 128, NSB, 512], bft,
                               kind="Internal").ap()
        out2_d = nc.dram_tensor("out2", [BPC, NGRP, NT, NSB, 512], bft,
                                kind="Internal").ap()
        tick_d = nc.declare_dram_parameter("tick", [128, 2], f32, isOutput=True)
    else:
        out_d = nc.declare_dram_parameter("out", [BPC, NGRP, 128, NSB, 512],
                                          bft, isOutput=True)
        out2_d = nc.declare_dram_parameter("out2", [BPC, NGRP, NT, NSB, 512],
                                           bft, isOutput=True)

    with tile.TileContext(nc) as tc:
        with (
            tc.tile_pool(name="const", bufs=1) as cp,
            tc.tile_pool(name="xp", bufs=2) as xpp,
            tc.tile_pool(name="tsb", bufs=2) as tsp,
            tc.tile_pool(name="vsb", bufs=2) as vsbp,
            tc.tile_pool(name="vstk", bufs=2) as vskp,
            tc.tile_pool(name="osb", bufs=2) as osbp,
            tc.tile_pool(name="rel", bufs=2) as relp,
            tc.tile_pool(name="vps", bufs=2, space="PSUM") as vpp,
            tc.tile_pool(name="fin", bufs=1, space="PSUM") as finp,
            tc.tile_pool(name="res", bufs=1, space="PSUM") as resp,
        ):
            wc_t = cp.tile([128, 2 * H], bft)
            th_t = cp.tile([128, 384], bft)
            rb_t = cp.tile([128, 512], bft)
            ln0_t = cp.tile([128, 128], bft)
            l2n0_t = cp.tile([128, 128], bft)
            ln1_t = cp.tile([128, NT], bft)
            l2n1_t = cp.tile([128, NT], bft)
            eyn0_t = cp.tile([128, 128], bft)
            lt2m_t = cp.tile([2 * NT, 128], bft)
            lt3t_t = cp.tile([3 * NT, NT], bft)
            cb_t = cp.tile([128, 2], f32)
            for t_, d_ in ((wc_t, wc_d), (th_t, th_d), (rb_t, rb_d),
                           (ln0_t, ln0_d), (l2n0_t, l2n0_d), (ln1_t, ln1_d),
                           (l2n1_t, l2n1_d), (eyn0_t, eyn0_d),
                           (lt2m_t, lt2m_d), (lt3t_t, lt3t_d), (cb_t, cb_d)):
                nc.sync.dma_start(out=t_[:, :], in_=d_[:, :])

            def st2a(st):
                """fin alloc + 4 full-width fm chain MMs."""
                sb, vsb, vstk, xp, t_sb, osb = st
                fin = finp.tile([128, 1024], f32, name="fin")
                st.append(fin)
                fm = fin[:, 0:512].rearrange("n (s c) -> n s c", s=8)
                nc.tensor.matmul(fm[:, :, :], ln0_t[:, :],
                                 vsb[:, :, 0, :, 64:128],
                                 start=True, stop=False, skip_group_check=True)
                nc.tensor.matmul(fm[:, :, :], l2n0_t[:, :],
                                 vsb[:, :, 0, :, 128:192],
                                 start=False, stop=False, skip_group_check=True)
                nc.tensor.matmul(fm[:, :, :], lt2m_t[0:2 * NT, :],
                                 vstk[0:2 * NT, :, :],
                                 start=False, stop=False, skip_group_check=True)
                nc.tensor.matmul(fm[:, :, :], eyn0_t[:, :],
                                 vsb[:, :, 0, :, 0:64],
                                 start=False, stop=True, skip_group_check=True)

            def st2b(st):
                """chain tail-half + residual + relu + add."""
                sb, vsb, vstk, xp, t_sb, osb, fin = st
                q0 = 4 * sb
                ft = fin[0:NT, 512:1024].rearrange("n (s c) -> n s c", s=8)
                nc.tensor.matmul(ft[:, :, :], ln1_t[:, 0:NT],
                                 vsb[:, :, 0, :, 64:128],
                                 start=True, stop=False, skip_group_check=True)
                nc.tensor.matmul(ft[:, :, :], l2n1_t[:, 0:NT],
                                 vsb[:, :, 0, :, 128:192],
                                 start=False, stop=False, skip_group_check=True)
                nc.tensor.matmul(ft[:, :, :], lt3t_t[0:3 * NT, 0:NT],
                                 vstk[0:3 * NT, :, :],
                                 start=False, stop=True, skip_group_check=True)
                res = resp.tile([128, 1024], f32, name="res")
                nc.tensor.matmul(res[:, 0:512], xp[0:31, q0, 0:128],
                                 rb_t[0:31, :], start=True, stop=True)
                nc.tensor.matmul(res[0:NT, 512:1024], xp[0:31, q0, 128:N],
                                 rb_t[0:31, :], start=True, stop=True)
                rel = relp.tile([128, 1024], bft, name="rel")
                nc.scalar.activation(rel[:, :], fin[:, :], Relu)
                nc.vector.tensor_tensor(osb[:, sb, :], rel[:, :],
                                        res[:, :], ADD)

            def body():
                for b in range(BPC):
                    for g in range(NGRP):
                        xp = xpp.tile([128, TC, N], bft, name="xp")
                        nc.sync.dma_start(out=xp[0:31, :, :],
                                          in_=x_d[b, g, :, :, :])
                        t_sb = tsp.tile([128, TC, N], bft, name="t_sb")
                        for jj in range(TC // 6):
                            craw = resp.tile([128, 1024], f32, name="res")
                            for jh in range(2):
                                j = 2 * jj + jh
                                cps = craw[:, 512 * jh:512 * jh + 510]
                                cps = cps.rearrange("n (t c) -> n t c", t=3)
                                nc.tensor.matmul(cps[:, :, :], wc_t[0:18, :],
                                                 xp[0:18, 3 * j:3 * j + 3, :],
                                                 start=True, stop=True)
                            cin = craw.rearrange(
                                "n (j c) -> n j c", j=2)[:, :, 0:510]
                            nc.scalar.activation(
                                t_sb[:, 6 * jj:6 * jj + 6, :].rearrange(
                                    "n t c -> n (t c)").rearrange(
                                    "n (j c) -> n j c", j=2),
                                cin, Relu, bias=cb_t[:, 0:1], scale=1.0)
                        osb = osbp.tile([128, NSB, 1024], bft, name="osb")
                        pend = None
                        for sb in range(NSB + 1):
                            if sb == NSB:
                                if pend is not None:
                                    st2a(pend)
                                    st2b(pend)
                                break
                            q0 = 4 * sb
                            vsb = vsbp.tile([128, 4, 2, 2, 192], bft,
                                            name="vsb")
                            vstk = vskp.tile([128, 8, 64], bft, name="vstk")
                            for p in range(4):
                                vps = vpp.tile([128, 2, 512], f32, name="vps")
                                nc.tensor.matmul(
                                    vps[:, 0, 0:384],
                                    t_sb[0:128, q0 + p, 0:128],
                                    th_t[0:128, :],
                                    start=True, stop=True)
                                nc.tensor.matmul(
                                    vps[0:NT, 1, 0:384],
                                    t_sb[0:128, q0 + p, 128:N],
                                    th_t[0:128, :],
                                    start=True, stop=True)
                                vv = vps[:, :, 0:384].rearrange(
                                    "n m (h c) -> n m h c", h=2)
                                if p % 2 == 0:
                                    nc.vector.tensor_copy(vsb[:, p, :, :, :],
                                                          vv[:, :, :, :])
                                else:
                                    nc.scalar.copy(vsb[:, p, :, :, :],
                                                   vv[:, :, :, :])
                                if p == 1 and pend is not None:
                                    st2a(pend)
                            # re-partition V-tail: [Vb_t; Vc_t; Va_t] stacked
                            # on partitions 0:126 for merged tail contracts
                            for ci, c0 in ((0, 64), (1, 128), (2, 0)):
                                for h in range(2):
                                    nc.gpsimd.dma_start(
                                        out=vstk[NT * ci:NT * ci + NT,
                                                 h::2, :],
                                        in_=vsb[0:NT, :, 1, h, c0:c0 + 64])
                            if pend is not None:
                                st2b(pend)
                            pend = [sb, vsb, vstk, xp, t_sb, osb]
                        nc.gpsimd.dma_start(
                            out=out_d[b, g, :, :, :],
                            in_=osb[:, :, 0:512])
                        nc.gpsimd.dma_start(
                            out=out2_d[b, g, :, :, :],
                            in_=osb[0:NT, :, 512:1024])

            if reps > 1:
                with tc.For_i(0, reps, 1):
                    body()
            else:
                body()
            if timing:
                nc.sync.dma_start(out=tick_d[:, :], in_=cb_t[:, :])

    nc.compile()
    return nc


def _prep(inputs):
    cheb = np.asarray(inputs["cheb"], np.float32)
    conv_w = np.asarray(inputs["conv_w"], np.float32)
    conv_b = np.asarray(inputs["conv_b"], np.float32)
    theta = np.asarray(inputs["theta"], np.float32)
    res_w = np.asarray(inputs["res_w"], np.float32)
    res_b = np.asarray(inputs["res_b"], np.float32)

    L = cheb[1]
    L2 = (cheb[2] + np.eye(N, dtype=np.float32)) / 2.0

    # block-diagonal conv weights (rows: [s1,s0,s2] x FIN, A cols 0:64,
    # B cols 64:128)
    wc = np.zeros((128, 2 * H), bf16)
    for slot, s in ((0, 1), (1, 0), (2, 2)):
        for f in range(FIN):
            wc[3 * slot + f, 0:H] = conv_w[:, f, 0, s].astype(bf16)
            wc[9 + 3 * slot + f, H:2 * H] = conv_w[:, f, 0, s].astype(bf16)

    # block-diagonal theta: rows 0:64 (A half) -> cols 0:192, rows 64:128
    # (B half) -> cols 192:384; col layout per half: [th0-th2 | th1 | 2*th2]
    th = np.zeros((128, 384), bf16)
    blk = np.concatenate([(theta[0] - theta[2]), theta[1],
                          2.0 * theta[2]], axis=1).astype(bf16)   # [64, 192]
    th[0:64, 0:192] = blk
    th[64:128, 192:384] = blk

    # residual rhs: fin col order slot=2p+h; center of pair p half A/B sits
    # at xp patch rows below; bias via ones row 30
    rb = np.zeros((128, 512), bf16)
    rwt = res_w.T.astype(bf16)
    rows = {(0, 0): 0, (0, 1): 9, (1, 0): 6, (1, 1): 15,
            (2, 0): 18, (2, 1): 24, (3, 0): 21, (3, 1): 27}
    for p in range(4):
        for h in range(2):
            slot = 2 * p + h
            r0 = rows[(p, h)]
            rb[r0:r0 + 3, 64 * slot:64 * slot + 64] = rwt
    for slot in range(8):
        rb[30, 64 * slot:64 * slot + 64] = res_b.astype(bf16)

    w = {
        "wc": wc, "th": th, "rb": rb,
        "ln0": L[0:128, 0:128].astype(bf16),
        "l2n0": L2[0:128, 0:128].astype(bf16),
        "ln1": L[0:128, 128:N].astype(bf16),
        "l2n1": L2[0:128, 128:N].astype(bf16),
        "eyn0": np.eye(128, dtype=bf16),
        "lt2m": np.concatenate(
            [L[128:N, 0:128], L2[128:N, 0:128]], axis=0).astype(bf16),
        "lt3t": np.concatenate(
            [L[128:N, 128:N], L2[128:N, 128:N],
             np.eye(NT, dtype=np.float32)], axis=0).astype(bf16),
    }
    cb = np.zeros((128, 2), np.float32)
    cb[0:64, 0] = conv_b
    cb[64:128, 0] = conv_b
    w["cb"] = cb
    return w


def _prep_x(x):
    """[B, FIN, N, T] -> [B, NGRP, 31, TC, N] bf16 per-group patch blocks.

    rows 0:9 A-half conv patch [s1, s0, s2] x FIN, rows 9:18 B-half,
    rows 18:24 [s3, s4] x FIN A-half, 24:30 B-half, row 30 ones.
    Zero-padded at global tau edges."""
    Bn = x.shape[0]
    xt = np.transpose(x, (0, 1, 3, 2))          # [B, FIN, T, N]
    xprep = np.zeros((Bn, 15, T, N), np.float32)
    xprep[:, 0:3] = xt                          # s1 = x[t]
    xprep[:, 3:6, 1:T] = xt[:, :, 0:T - 1]      # s0 = x[t-1]
    xprep[:, 6:9, 0:T - 1] = xt[:, :, 1:T]      # s2 = x[t+1]
    xprep[:, 9:12, 0:T - 2] = xt[:, :, 2:T]     # s3 = x[t+2]
    xprep[:, 12:15, 0:T - 3] = xt[:, :, 3:T]    # s4 = x[t+3]
    xall = np.zeros((Bn, NGRP, 31, TC, N), np.float32)
    xall[:, :, 30] = 1.0
    for g in range(NGRP):
        tbA = g * TC
        tbB = tbA + TH
        xall[:, g, 0:9] = xprep[:, 0:9, tbA:tbA + TC]
        xall[:, g, 9:18] = xprep[:, 0:9, tbB:tbB + TC]
        xall[:, g, 18:24] = xprep[:, 9:15, tbA:tbA + TC]
        xall[:, g, 24:30] = xprep[:, 9:15, tbB:tbB + TC]
    return xall.astype(bf16)


def _unshard_out(main, tail):
    """main [BPC, NGRP, 128, NSB, 512], tail [.., NT, ..] -> [BPC,O,N,T]"""
    out = np.empty((BPC, O, N, T), np.float32)
    main = np.asarray(main).astype(np.float32)
    tail = np.asarray(tail).astype(np.float32)
    for part, n0, n1 in ((main, 0, 128), (tail, 128, N)):
        r = part.reshape(BPC, NGRP, n1 - n0, NSB, 4, 2, O)
        # dims: b, g, n, sb, p, h, o ; tau = h*TH + g*TC + 4*sb + p
        r = np.transpose(r, (0, 6, 2, 5, 1, 3, 4))  # b, o, n, h, g, sb, p
        out[:, :, n0:n1, :] = r.reshape(BPC, O, n1 - n0, T)
    return out


def kernel(**inputs):
    from concourse.bass_utils import run_bass_kernel_spmd

    if "nc" not in _cache:
        _cache["nc"] = _build(1)
    nc = _cache["nc"]

    x = np.asarray(inputs["x"], np.float32)
    weights = _prep(inputs)
    xprep = _prep_x(x)
    in_maps = []
    for cid in range(NCORES):
        m = dict(weights)
        m["x"] = np.ascontiguousarray(xprep[cid * BPC:(cid + 1) * BPC])
        in_maps.append(m)
    res = run_bass_kernel_spmd(nc, in_maps, list(range(NCORES)), trace=False)
    out = np.concatenate(
        [_unshard_out(res.results[cid]["out"], res.results[cid]["out2"])
         for cid in range(NCORES)], axis=0)
    return out.astype(np.float32)
